# revision 16
# baseline (speedup 1.0000x reference)
"""Trainium2 Bass kernel for nn_MultiHeadAttention (B=2, T=2048, E=1024, H=16).

Contract: kernel(**inputs) takes FULL inputs (as produced by setup_inputs())
and returns the FULL output matching reference(): (output, attn_weights).

Sharding (8 cores, no collectives): core c handles batch b = c // 4 and query
rows q0 = (c % 4) * 512 .. q0+512 (all 16 heads).  K/V projections for the
full sequence are computed redundantly on each core of a batch group; Q
projection, scores, softmax, attn@V, out-proj, residual+LayerNorm are
computed for the core's own 512 query rows.  Host only slices inputs and
concatenates outputs.

Orientation notes (PE computes out = lhsT.T @ rhs, contraction on the
partition dim):
  - Q^T, K^T are built as (d, t) so scores S (q,k) and S^T (k,q) are single
    K=64 matmuls from SBUF-resident operands.
  - Softmax over k is done in the (q,k) orientation (reduce over the free
    dim); exp is evaluated WITHOUT max subtraction (scores for this problem
    are O(1); exp cannot overflow in fp32) so softmax(x) = exp(x/8)/sum.
  - attn@V needs k on partitions, so P^T = exp(S^T/8) is produced by a second
    score matmul + exp in the (k,q) orientation.  V gets a ones column
    appended so the attn@V matmul also yields the softmax denominator row in
    (1, q) layout for normalizing O^T.
  - Matmuls run in float32r (TF32-like, 1 cycle/row at free-dim >= 256);
    storage stays fp32 and PSUM accumulation is fp32.
"""

import numpy as np

import concourse.bacc as bacc
import concourse.mybir as mybir
import concourse.tile as tile
from concourse.bass_utils import run_bass_kernel_spmd

F32 = mybir.dt.float32
F32R = mybir.dt.float32r
AF = mybir.ActivationFunctionType
ALU = mybir.AluOpType

B, T, E, H, D = 2, 2048, 1024, 16, 64
NCORES = 8
QB = T // 4          # 512 query rows per core
NQT = QB // 128      # 4 q-tiles of 128
NKB = T // 512       # 4 key blocks of 512
NKT = T // 128       # 16 key tiles of 128
NE = E // 128        # 8 contraction chunks for projections
EPS = 1e-5
SCALE = 1.0 / 8.0    # 1/sqrt(D)

_nc_cache = {}


def r(ap):
    """float32r view of an fp32 AP (PE runs 4x faster on f32r)."""
    return ap.bitcast(F32R)


def build(rep=1, mm_dt="f32r"):
    """Build + compile the per-core program. rep>1 repeats the whole
    computation (same outputs) for wall-clock timing."""
    MDT = F32R if mm_dt == "f32r" else F32
    nc = bacc.Bacc("TRN2", target_bir_lowering=False, debug=False)

    xT = nc.dram_tensor("xT", [E, T], MDT, kind="ExternalInput")
    xTq = nc.dram_tensor("xTq", [E, QB], MDT, kind="ExternalInput")
    xq = nc.dram_tensor("xq", [QB, E], F32, kind="ExternalInput")
    wq = nc.dram_tensor("wq", [E, E], MDT, kind="ExternalInput")
    wk = nc.dram_tensor("wk", [E, E], MDT, kind="ExternalInput")
    wv = nc.dram_tensor("wv", [E, E], MDT, kind="ExternalInput")
    wo = nc.dram_tensor("wo", [E, E], MDT, kind="ExternalInput")
    bq = nc.dram_tensor("bq", [1, E], MDT, kind="ExternalInput")
    bk = nc.dram_tensor("bk", [1, E], MDT, kind="ExternalInput")
    bv = nc.dram_tensor("bv", [1, E], MDT, kind="ExternalInput")
    bo = nc.dram_tensor("bo", [1, E], MDT, kind="ExternalInput")
    gamma = nc.dram_tensor("gamma", [1, E], F32, kind="ExternalInput")
    beta = nc.dram_tensor("beta", [1, E], F32, kind="ExternalInput")
    out = nc.dram_tensor("out", [QB, E], F32, kind="ExternalOutput")
    attn = nc.dram_tensor("attn", [H, QB, T], F32, kind="ExternalOutput")

    with tile.TileContext(nc) as tc:
        with (
            tc.tile_pool(name="pers", bufs=1) as pers,
            tc.tile_pool(name="psAST", bufs=2, space="PSUM") as psA,
            tc.tile_pool(name="qkvp", bufs=2, space="PSUM") as qkvps,
            tc.tile_pool(name="psO", bufs=2, space="PSUM") as psO,
        ):
            psST = psA
            for _ in range(rep):
                _body(nc, tc, MDT, pers, qkvps, psA, psST, psO,
                      xT, xTq, xq, wq, wk, wv, wo, bq, bk, bv, bo,
                      gamma, beta, out, attn)
    nc.compile()
    return nc


def _body(nc, tc, MDT, pers, qkvps, psA, psST, psO,
          xT, xTq, xq, wq, wk, wv, wo, bq, bk, bv, bo, gamma, beta, out, attn):
    # ---------- persistent tiles ----------
    xT_sb = pers.tile([128, NE, T], MDT, tag="xT")        # 8.4 MB, full x^T
    QT_sb = pers.tile([128, NE, QB], MDT, tag="QT")       # 2.1 MB, Q^T (d, q)
    OT_acc = [pers.tile([128, QB], MDT, name=f"OT{cc}", tag=f"OT{cc}")
              for cc in range(NE)]                        # 2.1 MB, concat^T
    ones = pers.tile([1, 512], MDT, tag="ones")
    nc.vector.memset(ones.bitcast(F32) if ones.dtype == F32R else ones, 1.0)
    eps_t = pers.tile([128, 1], F32, tag="eps")
    nc.vector.memset(eps_t, EPS)
    one1 = pers.tile([1, 1], F32, tag="one1")
    nc.vector.memset(one1, 1.0)

    # ---------- phase 0: Q^T = (x_q @ wq + bq)^T, laid out (d, q) ----------
    with tc.tile_pool(name="ph0", bufs=1) as ph0:
        wq_sb = ph0.tile([128, NE, E], MDT, tag="wq")
        xTq_sb = ph0.tile([128, NE, QB], MDT, tag="xTq")
        for e in range(NE):
            nc.sync.dma_start(out=wq_sb[:, e, :], in_=wq[e * 128:(e + 1) * 128, :])
            nc.sync.dma_start(out=xTq_sb[:, e, :], in_=xTq[e * 128:(e + 1) * 128, :])
        bq_sb = ph0.tile([1, E], MDT, tag="bq")
        nc.sync.dma_start(out=bq_sb, in_=bq[:, :])
        for dt_ in range(NE):
            ps = qkvps.tile([128, QB], F32, name="psq", tag="qkv")
            for e in range(NE):
                nc.tensor.matmul(ps, (wq_sb[:, e, dt_ * 128:(dt_ + 1) * 128]),
                                 (xTq_sb[:, e, :]), start=(e == 0), stop=False)
            nc.tensor.matmul(ps, (bq_sb[0:1, dt_ * 128:(dt_ + 1) * 128]),
                             (ones[0:1, 0:QB]), start=False, stop=True)
            nc.vector.tensor_copy(out=QT_sb[:, dt_, :], in_=ps)

    for e in range(NE):
        nc.sync.dma_start(out=xT_sb[:, e, :], in_=xT[e * 128:(e + 1) * 128, :])

    # biases kept for the whole kernel (tiny)
    bk_sb = pers.tile([1, E], MDT, tag="bk")
    bv_sb = pers.tile([1, E], MDT, tag="bv")
    nc.sync.dma_start(out=bk_sb, in_=bk[:, :])
    nc.sync.dma_start(out=bv_sb, in_=bv[:, :])

    # ---------- head groups ----------
    # K^T in 2-head units (double-buffered so the next unit's projection
    # overlaps this unit's attention); V~ in 4-head groups.
    with (
        tc.tile_pool(name="wg", bufs=2) as wg,
        tc.tile_pool(name="grp", bufs=2) as grp,
        tc.tile_pool(name="vgrp", bufs=1) as vgrp,
        tc.tile_pool(name="hd", bufs=2) as hd,
        tc.tile_pool(name="pt", bufs=3) as ptp,
    ):
        for pair in range(8):
            c0 = pair * 128  # first channel of this 2-head unit
            wk_sb = wg.tile([128, NE, 128], MDT, tag="wk")
            for e in range(NE):
                nc.sync.dma_start(out=wk_sb[:, e, :],
                                  in_=wk[e * 128:(e + 1) * 128, c0:c0 + 128])
            # K^T unit: (128, k) holding d-cols c0..c0+128 (heads 2*pair, 2*pair+1)
            KT = grp.tile([128, T], MDT, tag="KT")
            for kb in range(NKB):
                ps = qkvps.tile([128, 512], F32, name="psq", tag="qkv")
                for e in range(NE):
                    nc.tensor.matmul(
                        ps, (wk_sb[:, e, :]),
                        (xT_sb[:, e, kb * 512:(kb + 1) * 512]),
                        start=(e == 0), stop=False)
                nc.tensor.matmul(
                    ps, (bk_sb[0:1, c0:c0 + 128]),
                    (ones[0:1, 0:512]), start=False, stop=True)
                nc.vector.tensor_copy(out=KT[:, kb * 512:(kb + 1) * 512], in_=ps)

            if pair % 2 == 0:
                # V~ for the 4-head group: (128, kt, h, 65); col 64 = 1.0
                g4 = pair // 2
                wv_sb = wg.tile([128, NE, 256], MDT, tag="wv", bufs=1)
                for e in range(NE):
                    nc.sync.dma_start(out=wv_sb[:, e, :],
                                      in_=wv[e * 128:(e + 1) * 128,
                                             g4 * 256:(g4 + 1) * 256])
                Vt = vgrp.tile([128, NKT, 4, 65], MDT, tag="Vt")
                vcol = Vt[:, :, :, 64:65]
                nc.vector.memset(vcol.bitcast(F32) if vcol.dtype == F32R else vcol, 1.0)
                for kt in range(NKT):
                    ps = qkvps.tile([128, 256], F32, name="psq", tag="qkv")
                    for e in range(NE):
                        nc.tensor.matmul(ps, (xT_sb[:, e, kt * 128:(kt + 1) * 128]),
                                         (wv_sb[:, e, :]), start=(e == 0), stop=False)
                    nc.tensor.matmul(ps, (ones[0:1, 0:128]),
                                     (bv_sb[0:1, g4 * 256:(g4 + 1) * 256]),
                                     start=False, stop=True)
                    nc.vector.tensor_copy(
                        out=Vt[:, kt, :, 0:64],
                        in_=ps.rearrange("p (h d) -> p h d", h=4))

            for h in range(2):
                hh = pair * 2 + h
                p0 = h * 64          # partition base inside the KT unit
                hv = hh % 4          # head index inside the V~ 4-head group
                sQ = hh // 2         # Q^T slot for this head

                # --- stage B/C first: P^T = exp(S^T), O^T = V~^T @ P^T ---
                Ops = psO.tile([65, QB], F32, tag="O")
                for kt2 in range(NKT // 2):
                    ps = psST.tile([128, 2, QB], F32, name="pst", tag="AST")
                    for i in range(2):
                        kt = kt2 * 2 + i
                        nc.tensor.matmul(
                            ps[:, i, :], (KT[p0:p0 + 64, kt * 128:(kt + 1) * 128]),
                            (QT_sb[p0:p0 + 64, sQ, :]),
                            start=True, stop=True)
                    PT = ptp.tile([128, 2, QB], MDT, tag="PT")
                    nc.scalar.activation(out=PT, in_=ps, func=AF.Exp, scale=SCALE)
                    for i in range(2):
                        kt = kt2 * 2 + i
                        nc.tensor.matmul(Ops, (Vt[:, kt, hv, :]), (PT[:, i, :]),
                                         start=(kt == 0), stop=(kt == NKT - 1))
                # normalize O^T rows by the denominator (row 64 of Ops)
                rec = hd.tile([1, QB], F32, tag="rec", bufs=2)
                nc.vector.reciprocal(out=rec, in_=Ops[64:65, :])
                bc = hd.tile([64, QB], F32, tag="bc", bufs=1)
                nc.gpsimd.partition_broadcast(bc, rec)
                Otmp = hd.tile([64, QB], MDT, tag="Otmp", bufs=1)
                nc.vector.tensor_mul(out=Otmp, in0=Ops[0:64, :], in1=bc)
                nc.sync.dma_start(out=OT_acc[sQ][p0:p0 + 64, :], in_=Otmp)

                # transpose rec (1, q) -> per-partition (q, 1) per q-tile,
                # via 4 tiny K=1/N=1 matmuls against a ones scalar
                rsp = psO.tile([128, NQT], F32, name="rsp", tag="O")
                for qt in range(NQT):
                    nc.tensor.matmul(rsp[:, qt:qt + 1],
                                     (rec[0:1, qt * 128:(qt + 1) * 128]),
                                     (one1[0:1, 0:1]), start=True, stop=True)
                rs4 = hd.tile([128, NQT], F32, tag="rs4", bufs=2)
                nc.vector.tensor_copy(out=rs4, in_=rsp)

                # --- stage A: S (q,k), P = exp(S)*rs, attn output ---
                for qt in range(NQT):
                    P = hd.tile([128, T], F32, tag="P", bufs=3)
                    for kb2 in range(NKB // 2):
                        ps = psA.tile([128, 2, 512], F32, name="psa", tag="AST")
                        for i in range(2):
                            kb = kb2 * 2 + i
                            nc.tensor.matmul(
                                ps[:, i, :],
                                (QT_sb[p0:p0 + 64, sQ, qt * 128:(qt + 1) * 128]),
                                (KT[p0:p0 + 64, kb * 512:(kb + 1) * 512]),
                                start=True, stop=True)
                        nc.scalar.activation(
                            out=P[:, kb2 * 1024:(kb2 + 1) * 1024].rearrange(
                                "p (b f) -> p b f", b=2),
                            in_=ps, func=AF.Exp, scale=SCALE)
                    nc.vector.tensor_scalar_mul(P, P, rs4[:, qt:qt + 1])
                    nc.sync.dma_start(out=attn[hh, qt * 128:(qt + 1) * 128, :], in_=P)

    # ---------- tail: out-proj + residual + LayerNorm ----------
    with (
        tc.tile_pool(name="tail", bufs=1) as tail,
        tc.tile_pool(name="lnw", bufs=2) as lnw,
    ):
        wo_sb = tail.tile([128, NE, E], MDT, tag="wo")
        for e in range(NE):
            nc.sync.dma_start(out=wo_sb[:, e, :], in_=wo[e * 128:(e + 1) * 128, :])
        bo_sb = tail.tile([1, E], MDT, tag="bo")
        nc.sync.dma_start(out=bo_sb, in_=bo[:, :])
        xq_sb = tail.tile([128, NQT, E], F32, tag="xq")
        for qt in range(NQT):
            nc.sync.dma_start(out=xq_sb[:, qt, :],
                              in_=xq[qt * 128:(qt + 1) * 128, :])
        gB = tail.tile([128, E], F32, tag="gB")
        bB = tail.tile([128, E], F32, tag="bB")
        nc.gpsimd.dma_start(out=gB, in_=gamma[0:1, :].to_broadcast([128, E]))
        nc.gpsimd.dma_start(out=bB, in_=beta[0:1, :].to_broadcast([128, E]))

        for qt in range(NQT):
            R = lnw.tile([128, E], F32, tag="R")
            for eb in range(2):
                ps = qkvps.tile([128, 512], F32, name="psy", tag="qkv")
                for cc in range(NE):
                    nc.tensor.matmul(
                        ps, (OT_acc[cc][:, qt * 128:(qt + 1) * 128]),
                        (wo_sb[:, cc, eb * 512:(eb + 1) * 512]),
                        start=(cc == 0), stop=False)
                nc.tensor.matmul(ps, (ones[0:1, 0:128]),
                                 (bo_sb[0:1, eb * 512:(eb + 1) * 512]),
                                 start=False, stop=True)
                nc.vector.tensor_add(out=R[:, eb * 512:(eb + 1) * 512],
                                     in0=ps, in1=xq_sb[:, qt, eb * 512:(eb + 1) * 512])
            # LayerNorm over the 1024 free elements
            stats = lnw.tile([128, 2, 6], F32, tag="stats")
            for sg in range(2):
                nc.vector.bn_stats(out=stats[:, sg, :],
                                   in_=R[:, sg * 512:(sg + 1) * 512])
            mv = lnw.tile([128, 2], F32, tag="mv")
            nc.vector.bn_aggr(out=mv, in_=stats)
            sd = lnw.tile([128, 1], F32, tag="sd")
            nc.scalar.activation(out=sd, in_=mv[:, 1:2], func=AF.Sqrt,
                                 bias=eps_t, scale=1.0)
            rstd = lnw.tile([128, 1], F32, tag="rstd")
            nc.vector.reciprocal(out=rstd, in_=sd)
            T1 = lnw.tile([128, E], F32, tag="T1")
            nc.vector.tensor_scalar(out=T1, in0=R, scalar1=mv[:, 0:1],
                                    scalar2=rstd, op0=ALU.subtract, op1=ALU.mult)
            T2 = lnw.tile([128, E], F32, tag="T2")
            nc.vector.tensor_mul(out=T2, in0=T1, in1=gB)
            Ofin = lnw.tile([128, E], F32, tag="Ofin")
            nc.vector.tensor_add(out=Ofin, in0=T2, in1=bB)
            nc.sync.dma_start(out=out[qt * 128:(qt + 1) * 128, :], in_=Ofin)


def make_in_maps(query, wq, bq, wk, bk, wv, bv, wo, bo, gamma, beta):
    in_maps = []
    xTs = [np.ascontiguousarray(query[b].T) for b in range(B)]
    row = lambda v: np.ascontiguousarray(v.reshape(1, E))
    shared = {
        "wq": np.ascontiguousarray(wq), "wk": np.ascontiguousarray(wk),
        "wv": np.ascontiguousarray(wv), "wo": np.ascontiguousarray(wo),
        "bq": row(bq), "bk": row(bk), "bv": row(bv), "bo": row(bo),
        "gamma": row(gamma), "beta": row(beta),
    }
    for c in range(NCORES):
        b, q0 = c // 4, (c % 4) * QB
        m = dict(shared)
        m["xT"] = xTs[b]
        m["xTq"] = np.ascontiguousarray(xTs[b][:, q0:q0 + QB])
        m["xq"] = np.ascontiguousarray(query[b, q0:q0 + QB, :])
        in_maps.append(m)
    return in_maps


def kernel(query, mask, wq, bq, wk, bk, wv, bv, wo, bo, gamma, beta):
    query = np.asarray(query, dtype=np.float32)
    mask = np.asarray(mask)
    assert np.all(mask != 0), "kernel compiled for the all-ones mask of this problem"
    args = [np.asarray(a, dtype=np.float32)
            for a in (wq, bq, wk, bk, wv, bv, wo, bo, gamma, beta)]

    if "nc" not in _nc_cache:
        _nc_cache["nc"] = build()
    nc = _nc_cache["nc"]

    in_maps = make_in_maps(query, *args)
    res = run_bass_kernel_spmd(nc, in_maps, core_ids=list(range(NCORES))).results

    output = np.empty((B, T, E), np.float32)
    attn_w = np.empty((B, H, T, T), np.float32)
    for c in range(NCORES):
        b, q0 = c // 4, (c % 4) * QB
        output[b, q0:q0 + QB, :] = res[c]["out"]
        attn_w[b, :, q0:q0 + QB, :] = res[c]["attn"]
    return output, attn_w


# revision 21
# speedup vs baseline: 1.1165x; 1.1165x over previous
"""Trainium2 Bass kernel for nn_MultiHeadAttention (B=2, T=2048, E=1024, H=16).

Contract: kernel(**inputs) takes FULL inputs (as produced by setup_inputs())
and returns the FULL output matching reference(): (output, attn_weights).

Sharding (8 cores, no collectives): core c handles batch b = c // 4 and query
rows q0 = (c % 4) * 512 .. q0+512 (all 16 heads).  K/V projections for the
full sequence are computed redundantly on each core of a batch group; Q
projection, scores, softmax, attn@V, out-proj, residual+LayerNorm are
computed for the core's own 512 query rows.  Host only slices inputs and
concatenates outputs.

Orientation notes (PE computes out = lhsT.T @ rhs, contraction on the
partition dim):
  - Q^T, K^T are built as (d, t) so scores S (q,k) and S^T (k,q) are single
    K=64 matmuls from SBUF-resident operands.
  - Softmax over k is done in the (q,k) orientation (reduce over the free
    dim); exp is evaluated WITHOUT max subtraction (scores for this problem
    are O(1); exp cannot overflow in fp32) so softmax(x) = exp(x/8)/sum.
  - attn@V needs k on partitions, so P^T = exp(S^T/8) is produced by a second
    score matmul + exp in the (k,q) orientation.  V gets a ones column
    appended so the attn@V matmul also yields the softmax denominator row in
    (1, q) layout for normalizing O^T.
  - Matmuls run in float32r (TF32-like, 1 cycle/row at free-dim >= 256);
    storage stays fp32 and PSUM accumulation is fp32.
"""

import numpy as np

import concourse.bacc as bacc
import concourse.mybir as mybir
import concourse.tile as tile
from concourse.bass_utils import run_bass_kernel_spmd

F32 = mybir.dt.float32
F32R = mybir.dt.float32r
AF = mybir.ActivationFunctionType
ALU = mybir.AluOpType

B, T, E, H, D = 2, 2048, 1024, 16, 64
NCORES = 8
QB = T // 4          # 512 query rows per core
NQT = QB // 128      # 4 q-tiles of 128
NKB = T // 512       # 4 key blocks of 512
NKT = T // 128       # 16 key tiles of 128
NE = E // 128        # 8 contraction chunks for projections
EPS = 1e-5
SCALE = 1.0 / 8.0    # 1/sqrt(D)

_nc_cache = {}


def r(ap):
    """float32r view of an fp32 AP (PE runs 4x faster on f32r)."""
    return ap.bitcast(F32R)


def build(rep=1, mm_dt="f32r"):
    """Build + compile the per-core program. rep>1 repeats the whole
    computation (same outputs) for wall-clock timing."""
    MDT = F32R if mm_dt == "f32r" else F32
    nc = bacc.Bacc("TRN2", target_bir_lowering=False, debug=False)

    xT = nc.dram_tensor("xT", [E, T], MDT, kind="ExternalInput")
    xTq = nc.dram_tensor("xTq", [E, QB], MDT, kind="ExternalInput")
    xq = nc.dram_tensor("xq", [QB, E], F32, kind="ExternalInput")
    wq = nc.dram_tensor("wq", [E, E], MDT, kind="ExternalInput")
    wk = nc.dram_tensor("wk", [E, E], MDT, kind="ExternalInput")
    wv = nc.dram_tensor("wv", [E, E], MDT, kind="ExternalInput")
    wo = nc.dram_tensor("wo", [E, E], MDT, kind="ExternalInput")
    bq = nc.dram_tensor("bq", [1, E], MDT, kind="ExternalInput")
    bk = nc.dram_tensor("bk", [1, E], MDT, kind="ExternalInput")
    bv = nc.dram_tensor("bv", [1, E], MDT, kind="ExternalInput")
    bo = nc.dram_tensor("bo", [1, E], MDT, kind="ExternalInput")
    gamma = nc.dram_tensor("gamma", [1, E], F32, kind="ExternalInput")
    beta = nc.dram_tensor("beta", [1, E], F32, kind="ExternalInput")
    out = nc.dram_tensor("out", [QB, E], F32, kind="ExternalOutput")
    attn = nc.dram_tensor("attn", [H, QB, T], F32, kind="ExternalOutput")

    with tile.TileContext(nc) as tc:
        with (
            tc.tile_pool(name="pers", bufs=1) as pers,
            tc.tile_pool(name="psAST", bufs=2, space="PSUM") as psA,
            tc.tile_pool(name="qkvp", bufs=2, space="PSUM") as qkvps,
            tc.tile_pool(name="psO", bufs=2, space="PSUM") as psO,
        ):
            psST = psA
            for _ in range(rep):
                _body(nc, tc, MDT, pers, qkvps, psA, psST, psO,
                      xT, xTq, xq, wq, wk, wv, wo, bq, bk, bv, bo,
                      gamma, beta, out, attn)
    nc.compile()
    return nc


def _body(nc, tc, MDT, pers, qkvps, psA, psST, psO,
          xT, xTq, xq, wq, wk, wv, wo, bq, bk, bv, bo, gamma, beta, out, attn):
    # ---------- persistent tiles ----------
    xT_sb = pers.tile([128, NE, T], MDT, tag="xT")        # 8.4 MB, full x^T
    QT_sb = pers.tile([128, NE, QB], MDT, tag="QT")       # 2.1 MB, Q^T (d, q)
    OT_acc = [pers.tile([128, QB], MDT, name=f"OT{cc}", tag=f"OT{cc}")
              for cc in range(NE)]                        # 2.1 MB, concat^T
    ones = pers.tile([1, 512], MDT, tag="ones")
    nc.vector.memset(ones.bitcast(F32) if ones.dtype == F32R else ones, 1.0)
    eps_t = pers.tile([128, 1], F32, tag="eps")
    nc.vector.memset(eps_t, EPS)
    one1 = pers.tile([1, 1], F32, tag="one1")
    nc.vector.memset(one1, 1.0)

    # ---------- phase 0: Q^T = (x_q @ wq + bq)^T, laid out (d, q) ----------
    with tc.tile_pool(name="ph0", bufs=1) as ph0:
        wq_sb = ph0.tile([128, NE, E], MDT, tag="wq")
        xTq_sb = ph0.tile([128, NE, QB], MDT, tag="xTq")
        for e in range(NE):
            nc.sync.dma_start(out=wq_sb[:, e, :], in_=wq[e * 128:(e + 1) * 128, :])
            nc.sync.dma_start(out=xTq_sb[:, e, :], in_=xTq[e * 128:(e + 1) * 128, :])
        bq_sb = ph0.tile([1, E], MDT, tag="bq")
        nc.sync.dma_start(out=bq_sb, in_=bq[:, :])
        for dt_ in range(NE):
            ps = qkvps.tile([128, QB], F32, name="psq", tag="qkv")
            for e in range(NE):
                nc.tensor.matmul(ps, (wq_sb[:, e, dt_ * 128:(dt_ + 1) * 128]),
                                 (xTq_sb[:, e, :]), start=(e == 0), stop=False)
            nc.tensor.matmul(ps, (bq_sb[0:1, dt_ * 128:(dt_ + 1) * 128]),
                             (ones[0:1, 0:QB]), start=False, stop=True)
            nc.vector.tensor_copy(out=QT_sb[:, dt_, :], in_=ps)

    for e in range(NE):
        nc.sync.dma_start(out=xT_sb[:, e, :], in_=xT[e * 128:(e + 1) * 128, :])

    # biases kept for the whole kernel (tiny)
    bk_sb = pers.tile([1, E], MDT, tag="bk")
    bv_sb = pers.tile([1, E], MDT, tag="bv")
    nc.sync.dma_start(out=bk_sb, in_=bk[:, :])
    nc.sync.dma_start(out=bv_sb, in_=bv[:, :])

    # ---------- head groups ----------
    # K^T in 2-head units (double-buffered so the next unit's projection
    # overlaps this unit's attention); V~ in 4-head groups.
    with (
        tc.tile_pool(name="wg", bufs=2) as wg,
        tc.tile_pool(name="grp", bufs=2) as grp,
        tc.tile_pool(name="vgrp", bufs=1) as vgrp,
        tc.tile_pool(name="hd", bufs=2) as hd,
        tc.tile_pool(name="pt", bufs=3) as ptp,
    ):
        for pair in range(8):
            c0 = pair * 128  # first channel of this 2-head unit
            wk_sb = wg.tile([128, NE, 128], MDT, tag="wk")
            for e in range(NE):
                nc.sync.dma_start(out=wk_sb[:, e, :],
                                  in_=wk[e * 128:(e + 1) * 128, c0:c0 + 128])
            # K^T unit: (128, k) holding d-cols c0..c0+128 (heads 2*pair, 2*pair+1)
            KT = grp.tile([128, T], MDT, tag="KT")
            for kb in range(NKB):
                ps = qkvps.tile([128, 512], F32, name="psq", tag="qkv")
                for e in range(NE):
                    nc.tensor.matmul(
                        ps, (wk_sb[:, e, :]),
                        (xT_sb[:, e, kb * 512:(kb + 1) * 512]),
                        start=(e == 0), stop=False)
                nc.tensor.matmul(
                    ps, (bk_sb[0:1, c0:c0 + 128]),
                    (ones[0:1, 0:512]), start=False, stop=True)
                nc.vector.tensor_copy(out=KT[:, kb * 512:(kb + 1) * 512], in_=ps)

            if pair % 2 == 0:
                # V~ for the 4-head group: (128, kt, h, 65); col 64 = 1.0
                g4 = pair // 2
                wv_sb = wg.tile([128, NE, 256], MDT, tag="wv", bufs=1)
                for e in range(NE):
                    nc.sync.dma_start(out=wv_sb[:, e, :],
                                      in_=wv[e * 128:(e + 1) * 128,
                                             g4 * 256:(g4 + 1) * 256])
                Vt = vgrp.tile([128, NKT, 4, 65], MDT, tag="Vt")
                vcol = Vt[:, :, :, 64:65]
                nc.vector.memset(vcol.bitcast(F32) if vcol.dtype == F32R else vcol, 1.0)
                for kt in range(NKT):
                    ps = qkvps.tile([128, 256], F32, name="psq", tag="qkv")
                    for e in range(NE):
                        nc.tensor.matmul(ps, (xT_sb[:, e, kt * 128:(kt + 1) * 128]),
                                         (wv_sb[:, e, :]), start=(e == 0), stop=False)
                    nc.tensor.matmul(ps, (ones[0:1, 0:128]),
                                     (bv_sb[0:1, g4 * 256:(g4 + 1) * 256]),
                                     start=False, stop=True)
                    nc.vector.tensor_copy(
                        out=Vt[:, kt, :, 0:64],
                        in_=ps.rearrange("p (h d) -> p h d", h=4))

            for h in range(2):
                hh = pair * 2 + h
                p0 = h * 64          # partition base inside the KT unit
                hv = hh % 4          # head index inside the V~ 4-head group
                sQ = hh // 2         # Q^T slot for this head

                # --- stage B/C first: P^T = exp(S^T), O^T = V~^T @ P^T ---
                Ops = psO.tile([65, QB], F32, tag="O")
                for kt2 in range(NKT // 2):
                    ps = psST.tile([128, 2, QB], F32, name="pst", tag="AST")
                    for i in range(2):
                        kt = kt2 * 2 + i
                        nc.tensor.matmul(
                            ps[:, i, :], (KT[p0:p0 + 64, kt * 128:(kt + 1) * 128]),
                            (QT_sb[p0:p0 + 64, sQ, :]),
                            start=True, stop=True)
                    PT = ptp.tile([128, 2, QB], MDT, tag="PT")
                    nc.scalar.activation(out=PT, in_=ps, func=AF.Exp, scale=SCALE)
                    for i in range(2):
                        kt = kt2 * 2 + i
                        nc.tensor.matmul(Ops, (Vt[:, kt, hv, :]), (PT[:, i, :]),
                                         start=(kt == 0), stop=(kt == NKT - 1))
                # normalize O^T rows by the denominator (row 64 of Ops)
                rec = hd.tile([1, QB], F32, tag="rec", bufs=2)
                nc.vector.reciprocal(out=rec, in_=Ops[64:65, :])
                bc = hd.tile([64, QB], F32, tag="bc", bufs=1)
                nc.gpsimd.partition_broadcast(bc, rec)
                Otmp = hd.tile([64, QB], MDT, tag="Otmp", bufs=1)
                nc.vector.tensor_mul(out=Otmp, in0=Ops[0:64, :], in1=bc)
                nc.sync.dma_start(out=OT_acc[sQ][p0:p0 + 64, :], in_=Otmp)

                # transpose rec (1, q) -> per-partition (q, 1) per q-tile,
                # via 4 tiny K=1/N=1 matmuls against a ones scalar
                rsp = psO.tile([128, NQT], F32, name="rsp", tag="O")
                for qt in range(NQT):
                    nc.tensor.matmul(rsp[:, qt:qt + 1],
                                     (rec[0:1, qt * 128:(qt + 1) * 128]),
                                     (one1[0:1, 0:1]), start=True, stop=True)
                rs4 = hd.tile([128, NQT], F32, tag="rs4", bufs=2)
                nc.vector.tensor_copy(out=rs4, in_=rsp)

                # --- stage A: S (q,k), P = exp(S)*rs, attn output ---
                for qt in range(NQT):
                    P = hd.tile([128, T], F32, tag="P", bufs=4)
                    for kb2 in range(NKB // 2):
                        ps = psA.tile([128, 2, 512], F32, name="psa", tag="AST")
                        for i in range(2):
                            kb = kb2 * 2 + i
                            nc.tensor.matmul(
                                ps[:, i, :],
                                (QT_sb[p0:p0 + 64, sQ, qt * 128:(qt + 1) * 128]),
                                (KT[p0:p0 + 64, kb * 512:(kb + 1) * 512]),
                                start=True, stop=True)
                        nc.scalar.activation(
                            out=P[:, kb2 * 1024:(kb2 + 1) * 1024].rearrange(
                                "p (b f) -> p b f", b=2),
                            in_=ps, func=AF.Exp, scale=SCALE)
                    nc.vector.tensor_scalar_mul(P, P, rs4[:, qt:qt + 1])
                    nc.sync.dma_start(out=attn[hh, qt * 128:(qt + 1) * 128, :], in_=P)

    # ---------- tail: out-proj + residual + LayerNorm ----------
    with (
        tc.tile_pool(name="tail", bufs=1) as tail,
        tc.tile_pool(name="lnw", bufs=2) as lnw,
    ):
        wo_sb = tail.tile([128, NE, E], MDT, tag="wo")
        for e in range(NE):
            nc.sync.dma_start(out=wo_sb[:, e, :], in_=wo[e * 128:(e + 1) * 128, :])
        bo_sb = tail.tile([1, E], MDT, tag="bo")
        nc.sync.dma_start(out=bo_sb, in_=bo[:, :])
        xq_sb = tail.tile([128, NQT, E], F32, tag="xq")
        for qt in range(NQT):
            nc.sync.dma_start(out=xq_sb[:, qt, :],
                              in_=xq[qt * 128:(qt + 1) * 128, :])
        gB = tail.tile([128, E], F32, tag="gB")
        bB = tail.tile([128, E], F32, tag="bB")
        nc.gpsimd.dma_start(out=gB, in_=gamma[0:1, :].to_broadcast([128, E]))
        nc.gpsimd.dma_start(out=bB, in_=beta[0:1, :].to_broadcast([128, E]))

        for qt in range(NQT):
            R = lnw.tile([128, E], F32, tag="R")
            for eb in range(2):
                ps = qkvps.tile([128, 512], F32, name="psy", tag="qkv")
                for cc in range(NE):
                    nc.tensor.matmul(
                        ps, (OT_acc[cc][:, qt * 128:(qt + 1) * 128]),
                        (wo_sb[:, cc, eb * 512:(eb + 1) * 512]),
                        start=(cc == 0), stop=False)
                nc.tensor.matmul(ps, (ones[0:1, 0:128]),
                                 (bo_sb[0:1, eb * 512:(eb + 1) * 512]),
                                 start=False, stop=True)
                nc.vector.tensor_add(out=R[:, eb * 512:(eb + 1) * 512],
                                     in0=ps, in1=xq_sb[:, qt, eb * 512:(eb + 1) * 512])
            # LayerNorm over the 1024 free elements
            stats = lnw.tile([128, 2, 6], F32, tag="stats")
            for sg in range(2):
                nc.vector.bn_stats(out=stats[:, sg, :],
                                   in_=R[:, sg * 512:(sg + 1) * 512])
            mv = lnw.tile([128, 2], F32, tag="mv")
            nc.vector.bn_aggr(out=mv, in_=stats)
            sd = lnw.tile([128, 1], F32, tag="sd")
            nc.scalar.activation(out=sd, in_=mv[:, 1:2], func=AF.Sqrt,
                                 bias=eps_t, scale=1.0)
            rstd = lnw.tile([128, 1], F32, tag="rstd")
            nc.vector.reciprocal(out=rstd, in_=sd)
            T1 = lnw.tile([128, E], F32, tag="T1")
            nc.vector.tensor_scalar(out=T1, in0=R, scalar1=mv[:, 0:1],
                                    scalar2=rstd, op0=ALU.subtract, op1=ALU.mult)
            T2 = lnw.tile([128, E], F32, tag="T2")
            nc.vector.tensor_mul(out=T2, in0=T1, in1=gB)
            Ofin = lnw.tile([128, E], F32, tag="Ofin")
            nc.vector.tensor_add(out=Ofin, in0=T2, in1=bB)
            nc.sync.dma_start(out=out[qt * 128:(qt + 1) * 128, :], in_=Ofin)


def make_in_maps(query, wq, bq, wk, bk, wv, bv, wo, bo, gamma, beta):
    in_maps = []
    xTs = [np.ascontiguousarray(query[b].T) for b in range(B)]
    row = lambda v: np.ascontiguousarray(v.reshape(1, E))
    shared = {
        "wq": np.ascontiguousarray(wq), "wk": np.ascontiguousarray(wk),
        "wv": np.ascontiguousarray(wv), "wo": np.ascontiguousarray(wo),
        "bq": row(bq), "bk": row(bk), "bv": row(bv), "bo": row(bo),
        "gamma": row(gamma), "beta": row(beta),
    }
    for c in range(NCORES):
        b, q0 = c // 4, (c % 4) * QB
        m = dict(shared)
        m["xT"] = xTs[b]
        m["xTq"] = np.ascontiguousarray(xTs[b][:, q0:q0 + QB])
        m["xq"] = np.ascontiguousarray(query[b, q0:q0 + QB, :])
        in_maps.append(m)
    return in_maps


def kernel(query, mask, wq, bq, wk, bk, wv, bv, wo, bo, gamma, beta):
    query = np.asarray(query, dtype=np.float32)
    mask = np.asarray(mask)
    assert np.all(mask != 0), "kernel compiled for the all-ones mask of this problem"
    args = [np.asarray(a, dtype=np.float32)
            for a in (wq, bq, wk, bk, wv, bv, wo, bo, gamma, beta)]

    if "nc" not in _nc_cache:
        _nc_cache["nc"] = build()
    nc = _nc_cache["nc"]

    in_maps = make_in_maps(query, *args)
    res = run_bass_kernel_spmd(nc, in_maps, core_ids=list(range(NCORES))).results

    output = np.empty((B, T, E), np.float32)
    attn_w = np.empty((B, H, T, T), np.float32)
    for c in range(NCORES):
        b, q0 = c // 4, (c % 4) * QB
        output[b, q0:q0 + QB, :] = res[c]["out"]
        attn_w[b, :, q0:q0 + QB, :] = res[c]["attn"]
    return output, attn_w
